# revision 11
# baseline (speedup 1.0000x reference)
"""Trainium2 Bass kernel for single-head decoder attention (v2).

Problem: B=8, S=2048, E=1024, D=128, O=1024 (fp32)
    q = query @ Wq + bq ; k = key @ Wk + bk ; v = value @ Wv + bv
    scores = (q @ k.T) / sqrt(D), causal-masked, softmax over keys
    out = (softmax @ v) @ Wo + bo
Sharding: data-parallel over batch, one batch element per NeuronCore.

v2 changes over the v1 pipeline (profiling showed v1 was DMA-launch-bound:
~100us/iter vs ~65us of PE work; one dma_start costs ~0.65us of queue time
and v1 issued 112 per iteration on a single HW queue):
  - Inputs land host-side in [128, 4, 8, 512] layout so each (tensor, group)
    is ONE 1-MiB dma_start with 8KB/partition contiguous lines: 12 input
    launches instead of 96, split across BOTH HW DGE queues (SP and ACT).
  - Output written per 512-row superblock as one [128, 4, 1024] SWDGE
    launch: 4 instead of 16.
  - Causal ranges tightened to 128-col granularity: score/PV/exp work on
    diagonal chunks starts at the diagonal (saves 3*3072 PE rows + the
    corresponding ACT elems); mask is one constant [128,128] triangle.
  - V projected directly into [seq, D] layout (stationary = x tile, moving
    = Wv) -- kills the 16 PE transposes of v1.
  - Softmax row-sums: exp'd tiles are pre-summed pairwise on DVE+Pool
    (engines idle vs PE) and ONE [128,512] matmul per superblock does the
    partition reduction: 4 matmuls instead of 40.
  - fp16 throughout (fp8-DoubleRow measured SLOWER than fp16 on this hw;
    fp32/fp32r matmuls pay a large per-instruction weight-load penalty).
  - bv and bo fold into one host-side bias added after gather (softmax rows
    sum to 1):  attn @ (V + 1 bv^T) @ Wo + bo = attn @ V @ Wo + (bv@Wo + bo).
  - scores computed TRANSPOSED: S_T[k, q] = kT_block.T @ qT_chunk, so the
    exp'd P_T[k, q] is directly the moving operand for the PV matmul with
    V tiles [seq, D] stationary; 1/rowsum commutes through the output
    projection and is applied on the HOST after gather (the [1,S] row-sum
    vector is DMA'd out) -- this removes the whole on-device reciprocal
    chain and decouples the output projection from the softmax sums.
"""

import numpy as np

import concourse.bacc as bacc
import concourse.mybir as mybir
import concourse.tile as tile
from concourse.bass_utils import run_bass_kernel_spmd

B, S, E, D, O = 8, 2048, 1024, 128, 1024
NCORES = 8
ET = E // 128          # 8 e-tiles
NSB = S // 512         # 4 q superblocks of 512
NQT = S // 128         # 16 q/k tiles of 128
SCALE = 1.0 / np.sqrt(D)
NEG = -1.0e30

F32 = mybir.dt.float32
DTYPE_MODE = "fp16"

_prog_cache: dict = {}


def _mdt(dtype_mode):
    return {
        "fp16": mybir.dt.float16,
        "bf16": mybir.dt.bfloat16,
    }[dtype_mode]


def _np_mdt(dtype_mode):
    import ml_dtypes
    return {
        "fp16": np.float16,
        "bf16": ml_dtypes.bfloat16,
    }[dtype_mode]


def _build(mode: str, dtype_mode: str, repeat: int = 1,
           no_in_dma: bool = False, no_out_dma: bool = False,
           no_compute: bool = False):
    """mode: 'causal' | 'full' | 'general'.

    repeat > 1 wraps the whole pipeline in a hardware For_i loop (same data
    each iteration) -- used only for steady-state timing measurements.
    no_in_dma/no_out_dma/no_compute are profiling-only ablations.
    """
    MDT = _mdt(dtype_mode)
    nc = bacc.Bacc("TRN2", target_bir_lowering=False, debug=False)

    # inputs pre-packed [p, group, e, s] so one dma_start covers a group
    xq = nc.dram_tensor("xq", [128, NSB, ET, 512], MDT, kind="ExternalInput").ap()
    xk = nc.dram_tensor("xk", [128, NSB, ET, 512], MDT, kind="ExternalInput").ap()
    xv = nc.dram_tensor("xv", [128, NSB, ET, 512], MDT, kind="ExternalInput").ap()
    wq = nc.dram_tensor("wq", [E, D], MDT, kind="ExternalInput").ap()
    wk = nc.dram_tensor("wk", [E, D], MDT, kind="ExternalInput").ap()
    wv = nc.dram_tensor("wv", [E, D], MDT, kind="ExternalInput").ap()
    wo = nc.dram_tensor("wo", [D, O], MDT, kind="ExternalInput").ap()
    bq = nc.dram_tensor("bq", [D, 1], F32, kind="ExternalInput").ap()
    bk = nc.dram_tensor("bk", [D, 1], F32, kind="ExternalInput").ap()
    ones = nc.dram_tensor("ones", [128, 1], MDT, kind="ExternalInput").ap()
    if mode == "causal":
        tri = nc.dram_tensor("tri", [128, 128], F32, kind="ExternalInput").ap()
    if mode == "general":
        biasT = nc.dram_tensor("biasT", [S, S], F32, kind="ExternalInput").ap()
    out = nc.dram_tensor("out", [S, O], MDT, kind="ExternalOutput").ap()
    rsout = nc.dram_tensor("rsout", [1, S], MDT, kind="ExternalOutput").ap()
    # per-superblock view: [sb][p, j, o] with q = sb*512 + j*128 + p
    out_sb_view = out.rearrange("(sb j p) o -> sb p j o", sb=NSB, j=4, p=128)

    Ident = mybir.ActivationFunctionType.Identity
    Exp = mybir.ActivationFunctionType.Exp

    def kmax_of(s):
        return 4 * s + 4 if mode == "causal" else NQT

    def lo_of(s, kj):
        """first valid q-col (within the superblock's 512) for k-tile kj."""
        if mode == "causal" and kj >= 4 * s:
            return (kj - 4 * s) * 128
        return 0

    with tile.TileContext(nc) as tc:
        with (
            tc.tile_pool(name="const", bufs=1) as const,
            tc.tile_pool(name="pers", bufs=1) as pers,
            tc.tile_pool(name="ptp", bufs=28) as ptp,
            tc.tile_pool(name="xstage", bufs=2) as xstage,
            tc.tile_pool(name="rsst", bufs=2) as rsst,
            tc.tile_pool(name="outst", bufs=2) as outst,
            tc.tile_pool(name="bstage", bufs=4) as bstage,
            tc.tile_pool(name="ps_big", bufs=2, space="PSUM") as ps_big,
            tc.tile_pool(name="ps_ot", bufs=2, space="PSUM") as ps_ot,
            tc.tile_pool(name="ps_sm", bufs=1, space="PSUM") as ps_sm,
        ):
            # ---- constants (outside the repeat loop) ----
            wq_sb = const.tile([128, ET, D], MDT)
            wk_sb = const.tile([128, ET, D], MDT)
            wv_sb = const.tile([128, ET, D], MDT)
            for w_sb, w_ap in ((wq_sb, wq), (wk_sb, wk), (wv_sb, wv)):
                nc.sync.dma_start(out=w_sb, in_=w_ap.rearrange("(e p) d -> p e d", p=128))
            wo_sb = const.tile([128, O], MDT)
            nc.sync.dma_start(out=wo_sb, in_=wo)
            bq_sb = const.tile([D, 1], F32)
            nc.sync.dma_start(out=bq_sb, in_=bq)
            bk_sb = const.tile([D, 1], F32)
            nc.sync.dma_start(out=bk_sb, in_=bk)
            ones_sb = const.tile([128, 1], MDT)
            nc.sync.dma_start(out=ones_sb, in_=ones)
            if mode == "causal":
                tri_sb = const.tile([128, 128], F32)
                nc.sync.dma_start(out=tri_sb, in_=tri)

            # ---- persistent tensors ----
            qT = pers.tile([D, S], MDT)
            kT = pers.tile([D, S], MDT)
            v_all = pers.tile([128, NQT, D], MDT)   # [seq%128, kj, D]
            oT = pers.tile([D, S], MDT)             # unnormalized (attn@V).T
            rs_sb = pers.tile([1, S], MDT)          # softmax row sums

            # ---- pipeline pieces ----
            if no_in_dma:
                xq_c = const.tile([128, ET, 512], MDT)
                xk_c = const.tile([128, ET, 512], MDT)
                xv_c = const.tile([128, ET, 512], MDT)
                nc.sync.dma_start(out=xq_c, in_=xq[:, 0, :, :])
                nc.sync.dma_start(out=xk_c, in_=xk[:, 0, :, :])
                nc.sync.dma_start(out=xv_c, in_=xv[:, 0, :, :])

            def emit_in_dma(n):
                """One 1-MiB dma_start per (tensor, group); q+half of v on
                SP, k+other half of v on ACT."""
                if no_in_dma:
                    return xq_c, xk_c, xv_c
                xq_t = xstage.tile([128, ET, 512], MDT, tag="xq", name="xq_t")
                xk_t = xstage.tile([128, ET, 512], MDT, tag="xk", name="xk_t")
                xv_t = xstage.tile([128, ET, 512], MDT, tag="xv", name="xv_t")
                nc.sync.dma_start(out=xq_t, in_=xq[:, n, :, :])
                nc.scalar.dma_start(out=xk_t, in_=xk[:, n, :, :])
                veng = nc.sync if n % 2 == 0 else nc.scalar
                veng.dma_start(out=xv_t, in_=xv[:, n, :, :])
                return xq_t, xk_t, xv_t

            def emit_proj_qk(n, xq_t, xk_t):
                csl = slice(n * 512, (n + 1) * 512)
                for x_t, w_sb, dest, b_sb, evq in (
                        (xq_t, wq_sb, qT, bq_sb, "act"),
                        (xk_t, wk_sb, kT, bk_sb, "dve")):
                    chunk = ps_big.tile([128, 512], F32, tag="pjc", name="pj")
                    for e in range(ET):
                        nc.tensor.matmul(
                            chunk, w_sb[:, e, :], x_t[:, e, :],
                            start=(e == 0), stop=(e == ET - 1))
                    # bias add fused into the eviction (ACT for q, DVE for
                    # k -- ACT is the busiest engine; GPSIMD can't read PSUM)
                    if evq == "act":
                        nc.scalar.activation(
                            out=dest[:, csl], in_=chunk, func=Ident,
                            bias=b_sb, scale=1.0)
                    else:
                        nc.vector.tensor_scalar_add(dest[:, csl], chunk, b_sb)

            def emit_proj_v(n, xv_t):
                """V in [seq, D] layout directly: stationary = x e/s-tile,
                moving = Wv e-tile; accumulate over e."""
                v_ps = ps_sm.tile([128, 4, D], F32, tag="sm", name="v_ps")
                for j in range(4):
                    for e in range(ET):
                        nc.tensor.matmul(
                            v_ps[:, j, :],
                            xv_t[:, e, j * 128:(j + 1) * 128],
                            wv_sb[:, e, :],
                            start=(e == 0), stop=(e == ET - 1))
                nc.vector.tensor_copy(v_all[:, 4 * n:4 * n + 4, :], v_ps)

            def emit_scores(s):
                """S_T + exp chunks for superblock s; returns pts list."""
                kmax = kmax_of(s)
                qs = qT[:, s * 512:(s + 1) * 512]
                pts = []
                for kj in range(kmax):
                    lo = lo_of(s, kj)
                    st = ps_big.tile([128, 512], F32, tag="st", bufs=3, name="st")
                    nc.tensor.matmul(
                        st[:, lo:], kT[:, kj * 128:(kj + 1) * 128], qs[:, lo:],
                        start=True, stop=True)
                    if mode == "causal" and kj >= 4 * s:
                        nc.vector.tensor_add(
                            st[:, lo:lo + 128], st[:, lo:lo + 128], tri_sb)
                    elif mode == "general":
                        bt = bstage.tile([128, 512], F32, tag="bias", name="bt")
                        nc.scalar.dma_start(
                            out=bt,
                            in_=biasT[kj * 128:(kj + 1) * 128,
                                      s * 512:(s + 1) * 512])
                        nc.vector.tensor_add(st, st, bt)
                    pt = ptp.tile([128, 512], MDT, tag="pt", name="pt")
                    nc.scalar.activation(
                        out=pt[:, lo:], in_=st[:, lo:], func=Exp, scale=SCALE)
                    pts.append(pt)
                return pts

            def emit_rowsums(s, pts):
                """Pre-sum exp'd tiles elementwise on DVE+Pool, then one
                matmul does the 128-partition reduction."""
                kmax = kmax_of(s)
                accD = rsst.tile([128, 512], MDT, tag="accD", name="accD")
                nc.gpsimd.tensor_copy(accD, pts[0])  # kj=0 is always full
                accP = None
                if kmax > 1:
                    accP = rsst.tile([128, 512], MDT, tag="accP", name="accP")
                    nc.gpsimd.memset(accP, 0.0)
                for kj in range(1, kmax):
                    lo = lo_of(s, kj)
                    eng, acc = ((nc.vector, accD) if kj % 2 == 0
                                else (nc.gpsimd, accP))
                    nc_add = eng.tensor_add
                    nc_add(acc[:, lo:], acc[:, lo:], pts[kj][:, lo:])
                if accP is not None:
                    nc.vector.tensor_add(accD, accD, accP)
                rs_ps = ps_sm.tile([1, 512], F32, tag="sm", name="rs_ps")
                nc.tensor.matmul(rs_ps, ones_sb, accD, start=True, stop=True)
                nc.vector.tensor_copy(rs_sb[:, s * 512:(s + 1) * 512], rs_ps)

            def emit_ot(s, pts):
                kmax = kmax_of(s)
                ot_ps = ps_ot.tile([128, 512], F32, tag="ot", name="ot_ps")
                for kj in range(kmax):
                    lo = lo_of(s, kj)
                    nc.tensor.matmul(
                        ot_ps[:, lo:], v_all[:, kj, :], pts[kj][:, lo:],
                        start=(kj == 0), stop=(kj == kmax - 1),
                        skip_group_check=True)
                nc.vector.tensor_copy(oT[:, s * 512:(s + 1) * 512], ot_ps)

            def emit_c(s):
                """Output projection + one batched SWDGE store for the 4
                q-tiles of superblock s."""
                ob = outst.tile([128, 4, O], MDT, tag="ob", name="ob")
                for j in range(4):
                    i = 4 * s + j
                    p0 = ps_big.tile([128, 512], F32, tag="pjc", name="c0")
                    p1 = ps_big.tile([128, 512], F32, tag="pjc", name="c1")
                    lhs = oT[:, i * 128:(i + 1) * 128]
                    nc.tensor.matmul(p0, lhs, wo_sb[:, :512], start=True, stop=True)
                    nc.tensor.matmul(p1, lhs, wo_sb[:, 512:], start=True, stop=True)
                    nc.vector.tensor_copy(ob[:, j, :512], p0)
                    nc.vector.tensor_copy(ob[:, j, 512:], p1)
                if not no_out_dma:
                    nc.gpsimd.dma_start(out=out_sb_view[s], in_=ob)

            def emit_pipeline():
                if no_compute:
                    # DMA skeleton only (profiling): stream inputs, write
                    # dummy outputs with the real launch pattern.
                    ob_c = const.tile([128, 4, O], MDT)
                    nc.vector.memset(ob_c, 0.25)
                    rs_c = const.tile([1, S], MDT)
                    nc.vector.memset(rs_c, 1.0)
                    for s in range(NSB):
                        emit_in_dma(s)
                        if not no_out_dma:
                            nc.gpsimd.dma_start(out=out_sb_view[s], in_=ob_c)
                    if not no_out_dma:
                        nc.sync.dma_start(out=rsout, in_=rs_c)
                    return
                if mode == "causal":
                    # superblock s needs qT/kT cols < (s+1)*512 and V tiles
                    # <= 4s+3 only, so attention interleaves with projection
                    # groups. C is deferred one superblock so its SWDGE
                    # store queues behind the next group's input DMAs.
                    xq_t, xk_t, xv_t = emit_in_dma(0)
                    for s in range(NSB):
                        nxt = emit_in_dma(s + 1) if s + 1 < NSB else None
                        emit_proj_qk(s, xq_t, xk_t)
                        pts = emit_scores(s)
                        emit_proj_v(s, xv_t)
                        emit_ot(s, pts)
                        if s > 0:
                            emit_c(s - 1)
                        emit_rowsums(s, pts)
                        if nxt is not None:
                            xq_t, xk_t, xv_t = nxt
                    emit_c(NSB - 1)
                    if not no_out_dma:
                        nc.sync.dma_start(out=rsout, in_=rs_sb)
                else:
                    xq_t, xk_t, xv_t = emit_in_dma(0)
                    stages = []
                    for n in range(NSB):
                        nxt = emit_in_dma(n + 1) if n + 1 < NSB else None
                        emit_proj_qk(n, xq_t, xk_t)
                        emit_proj_v(n, xv_t)
                        if nxt is not None:
                            xq_t, xk_t, xv_t = nxt
                    for s in range(NSB):
                        pts = emit_scores(s)
                        emit_ot(s, pts)
                        emit_rowsums(s, pts)
                        if s > 0:
                            emit_c(s - 1)
                    emit_c(NSB - 1)
                    if not no_out_dma:
                        nc.sync.dma_start(out=rsout, in_=rs_sb)

            import contextlib
            loop_cm = (tc.For_i(0, repeat, 1) if repeat > 1
                       else contextlib.nullcontext())
            with loop_cm:
                emit_pipeline()

    nc.compile()
    return nc


def _get_program(mode: str, dtype_mode: str, repeat: int = 1):
    key = (mode, dtype_mode, repeat)
    if key not in _prog_cache:
        _prog_cache[key] = _build(mode, dtype_mode, repeat)
    return _prog_cache[key]


def _tri_neg() -> np.ndarray:
    """tri[k, c] = 0 if c >= k else -1e30   (shape [128, 128])"""
    k = np.arange(128)[:, None]
    c = np.arange(128)[None, :]
    return np.where(c >= k, 0.0, NEG).astype(np.float32)


def build_in_maps(inputs: dict, mode: str, dtype_mode: str):
    """Host-side layout prep shared by kernel() and the test harness."""
    ndt = _np_mdt(dtype_mode)
    query = np.asarray(inputs["query"], dtype=np.float32)
    key = np.asarray(inputs["key"], dtype=np.float32)
    value = np.asarray(inputs["value"], dtype=np.float32)

    def pack(x):
        # [B, S, E] -> xT [B, E, S] -> [B, 128p, 4g, 8e, 512s]
        xT = x.transpose(0, 2, 1).reshape(B, ET, 128, NSB, 512)
        return np.ascontiguousarray(xT.transpose(0, 2, 3, 1, 4)).astype(ndt)

    xqP, xkP, xvP = pack(query), pack(key), pack(value)
    common = {
        "wq": np.asarray(inputs["Wq"], np.float32).astype(ndt),
        "wk": np.asarray(inputs["Wk"], np.float32).astype(ndt),
        "wv": np.asarray(inputs["Wv"], np.float32).astype(ndt),
        "wo": np.asarray(inputs["Wo"], np.float32).astype(ndt),
        "bq": np.asarray(inputs["bq"], np.float32).reshape(D, 1),
        "bk": np.asarray(inputs["bk"], np.float32).reshape(D, 1),
        "ones": np.ones((128, 1), np.float32).astype(ndt),
    }
    if mode == "causal":
        common["tri"] = _tri_neg()
    if mode == "general":
        mask2 = (np.asarray(inputs["mask"]).reshape(-1, S, S)[0] != 0)
        common["biasT"] = np.ascontiguousarray(
            np.where(mask2, 0.0, NEG).astype(np.float32).T)
    return [{**common, "xq": xqP[b], "xk": xkP[b], "xv": xvP[b]}
            for b in range(B)]


def detect_mode(mask) -> str:
    mask2 = (np.asarray(mask).reshape(-1, S, S)[0] != 0)
    if np.array_equal(mask2, np.tril(np.ones((S, S), dtype=bool))):
        return "causal"
    if mask2.all():
        return "full"
    return "general"


def kernel(**inputs) -> np.ndarray:
    mode = detect_mode(inputs["mask"])
    nc = _get_program(mode, DTYPE_MODE)
    in_maps = build_in_maps(inputs, mode, DTYPE_MODE)

    bv = np.asarray(inputs["bv"], dtype=np.float32)
    bo = np.asarray(inputs["bo"], dtype=np.float32)
    Wo = np.asarray(inputs["Wo"], dtype=np.float32)
    bo_eff = (bv.astype(np.float64) @ Wo.astype(np.float64) + bo).astype(np.float32)

    try:
        res = run_bass_kernel_spmd(nc, in_maps, list(range(NCORES)))
    except Exception:
        # transient NRT/terminal failures have been observed to clear on retry
        import time as _time
        _time.sleep(20)
        res = run_bass_kernel_spmd(nc, in_maps, list(range(NCORES)))
    outs = np.stack(
        [np.asarray(res.results[b]["out"], dtype=np.float32) for b in range(B)],
        axis=0)
    rs = np.stack(
        [np.asarray(res.results[b]["rsout"], dtype=np.float32)[0]
         for b in range(B)], axis=0)
    outs /= (rs[:, :, None] + 1e-30)
    outs += bo_eff[None, None, :]
    if mode == "general":
        # bv-folding assumes softmax rows sum to 1; fully-masked rows produce
        # all-zero attention (reference nan_to_num) and get only bo.
        mask2 = (np.asarray(inputs["mask"]).reshape(-1, S, S)[0] != 0)
        fully_masked = ~mask2.any(axis=1)
        if fully_masked.any():
            outs[:, fully_masked, :] = bo
    return outs.astype(np.float32)
